# revision 19
# baseline (speedup 1.0000x reference)
"""Trainium2 Bass kernel for nn_AlignModel.

Computes out[b, j, i] = sigmoid(simp[b,j]·w_s + orig[b,i]·w_o + bias) where
orig/simp are the two halves of prop_state[b] ([B, 2S, D] -> [B,S,D] each),
w_o = W[0,:D], w_s = W[0,D:].

Sharding: data-parallel over batch B=8 across the 8 NeuronCores. Each core:
  in  x   [4096, 512] f32  (= prop_state[b])
  in  w   [1, 1024]   f32
  in  bvec[1, 1]      f32
  out out [2048, 2048] fp16 (= sigmoid(s_s[:,None] + s_o[None,:] + b)),
  upcast to f32 on the host (tolerance is 2e-2; fp16 adds ~2^-11 rel).

Structure (v5).  Measured facts driving it: HBM reads sustain ~390 GB/s
once ramped but each DMA issue costs ~0.6-1 us on its queue's engine;
concurrent DMAs in a queue drain round-robin (chunk completion order is
by SIZE, not issue order); ScalarE sigmoid is 1 elem/cycle @1.2 GHz so
the 16-tile sigmoid train is a fixed ~32 us tail; DMA completion
semaphores cost extra microseconds when the consumer is another queue's
engine.

  - fp16 output halves store traffic (8.4 MB in + 8.4 MB out).
  - Critical path: orig half loaded -> s_o -> PSUM broadcast -> sigmoid
    train.  Orig loads use INCREASING chunk sizes [1,1,2,3,4,5] so
    round-robin completion is staggered (the dot pipeline starts on
    chunk 0 early) and are split across two queues (Sync 4 issues,
    Scalar 2) so all streams are live ~3 us sooner.
  - Dots: Pool (gpsimd) does the elementwise multiply, DVE the row
    reduce -- each leg is under the ~0.8 us/tile load pace, so s_o
    completes ~1.4 us after the last orig tile lands.  (Fallback
    POOL_MUL=False: DVE muls + ScalarE activation-Copy accum for early
    tiles, DVE reduces for the last chunk.)
  - s_o_mat [128,16] -> PE transpose (via identity) -> [16,128] psum ->
    DVE copy to fp16 t16 -> PSUM broadcast by SIXTEEN 128-wide rank-1
    matmuls whose moving operand is row n of t16 directly.  No DMA on
    this chain (a DMA hop here repeatedly cost 4+ us in completion
    semaphore latency).  The bias b is PSUM-seeded early with matching
    128-wide regions (start=True; the s_o matmuls close with stop=True).
  - simp loads are WAW-gated to release right as the orig stream
    finishes; simp dots (same Pool/DVE split) stay ahead of the sigmoid
    train.
  - Each output row-tile is ONE ScalarE op
      out_t = Sigmoid(psum(b + s_o) + s_s_col_t)   (PSUM -> SBUF fp16);
    row-tiles 0 and 15 ship as single 0.5 MiB stores, the rest as 1 MiB
    pairs.
  (tensor_tensor_reduce would fuse mul+reduce in one DVE pass but
  crashes the exec unit on real TRN2 despite passing CoreSim.)
"""

import numpy as np

import concourse.mybir as mybir
from concourse import bacc, bass_utils, masks
from concourse.tile import TileContext

P = 128          # partitions
D = 512          # feature dim
S = 2048         # sents
NT = S // P      # 16 tiles per half
# orig chunks as (size, start_tile): [1,1,2,4] per queue, staggered
OCHUNKS = [(1, 0), (1, 1), (2, 2), (4, 4),
           (1, 8), (1, 9), (2, 10), (4, 12)]
SCHUNKS = [1, 3, 4, 4, 4]      # simp tiles per chunk
NCORES = 8
F32 = mybir.dt.float32
F16 = mybir.dt.float16
# NOTE: Pool (gpsimd) tensor_mul and DVE tensor_tensor_reduce both crash
# the exec unit on real TRN2 inside this kernel (they pass CoreSim and
# work standalone) -- only memset/affine_select/DMA run on Pool here.


def _kernel_body(tc, out, x, w, bvec, sel_in):
    nc = tc.nc
    # both halves partition-inner: row index = n*P + p
    xo_re = x[0:S, :].rearrange("(n p) d -> p n d", p=P)
    xs_re = x[S:2 * S, :].rearrange("(n p) d -> p n d", p=P)
    with (
        tc.tile_pool(name="consts", bufs=1) as cpool,
        tc.tile_pool(name="xin", bufs=1) as xpool,
        tc.tile_pool(name="scratch", bufs=3) as spool,
        tc.tile_pool(name="outbuf", bufs=4) as opool,
        tc.tile_pool(name="psum", bufs=1, space="PSUM") as ppool,
    ):
        # b first: 4 bytes, completes immediately, unblocks the PSUM b-seed
        b_sb = cpool.tile([1, 1], F32, tag="bsb")
        nc.sync.dma_start(out=b_sb, in_=bvec)

        # --- orig input stream: [1,1,2,4] tiles per chunk on EACH of the
        # Sync (t0-7) and Scalar (t8-15) queues.  The DMA fabric is one
        # ~390 GB/s pool shared by all queues, but two queues issue their
        # chunks in parallel (each DMA_DIRECT2D costs ~0.6-1us of its
        # engine's sequencer), and balanced bytes per queue keep the
        # staggered completion order intact. ---
        xo_tiles = []
        for c, (sz, n0) in enumerate(OCHUNKS):
            xo = xpool.tile([P, sz, D], F32, tag=f"xo{c}", name=f"xo{c}")
            q = nc.sync if n0 < 8 else nc.scalar
            q.dma_start(out=xo, in_=xo_re[:, n0:n0 + sz, :])
            xo_tiles.append(xo)

        # simp tiles; loads issued later on the Sync queue, WAW-gated
        xs_tiles = [
            xpool.tile([P, sz, D], F32, tag=f"xs{g}", name=f"xs{g}")
            for g, sz in enumerate(SCHUNKS)
        ]

        # w / sel replicated via the Pool queue (idle); w_o first since it
        # gates the first mul.
        w_bc = cpool.tile([P, 2 * D], F32, tag="wbc")
        nc.gpsimd.dma_start(out=w_bc[:, 0:D],
                            in_=w[:, 0:D].broadcast_to([P, D]))
        nc.gpsimd.dma_start(out=w_bc[:, D:2 * D],
                            in_=w[:, D:2 * D].broadcast_to([P, D]))
        sel = cpool.tile([NT, S], F16, tag="sel")
        nc.gpsimd.dma_start(out=sel, in_=sel_in)
        ones_row = cpool.tile([1, P], F16, tag="ones")
        nc.gpsimd.memset(ones_row, 1.0)
        identity = cpool.tile([P, P], F32, tag="ident")
        masks.make_identity(nc, identity)

        s_o_mat = cpool.tile([P, NT], F32, tag="somat")   # s_o[n*128+p] @ [p,n]
        s_sb_mat = cpool.tile([P, NT], F32, tag="ssmat")  # s_s col t
        t16 = cpool.tile([NT, P], F16, tag="t16")         # s_o transposed
        b_row = cpool.tile([1, 512], F16, tag="brow")
        nc.vector.memset(b_row, 0.0)
        nc.vector.tensor_scalar_add(b_row, b_row, b_sb)
        sob_psum = ppool.tile([P, S], F32, tag="sob")     # b + s_o, all rows
        tp_psum = ppool.tile([NT, P], F32, tag="tp")      # transpose scratch

        # dummy sigmoid: pulls the ACT table load off the critical path
        dummy = cpool.tile([1, 2], F32, tag="dummy")
        nc.vector.memset(dummy, 0.0)
        nc.scalar.activation(dummy[:, 1:2], dummy[:, 0:1],
                             mybir.ActivationFunctionType.Sigmoid)

        # --- psum b seed: one start=True matmul per PSUM bank (512 cols;
        # accumulation groups are bank-granular) ---
        for j in range(S // 512):
            nc.tensor.matmul(sob_psum[:, j * 512:(j + 1) * 512], ones_row,
                             b_row, start=True, stop=False)

        # --- phase 1a: orig half -> s_o.  ONE DVE mul per chunk (w
        # zero-stride-broadcast along the tile axis) into an fp16 chunk
        # product; ScalarE -- idle here -- accumulates the small chunks'
        # tiles via activation-Copy accum (~1.0us/tile), DVE sweeps each
        # 4-tile chunk with a single chunked reduce. ---
        prods = {}
        mul_order = [0, 4, 1, 5, 2, 6, 3, 7]
        gates_after = {3: [0], 7: [1, 2]}       # chunk idx -> simp groups
        for c in mul_order:
            sz, n0 = OCHUNKS[c]
            xo = xo_tiles[c]
            prod = spool.tile([P, sz, D], F16, tag=f"prod{sz}",
                              name=f"po{c}")
            nc.vector.tensor_mul(out=prod, in0=xo,
                                 in1=w_bc[:, 0:D].rearrange(
                                     "p (o d) -> p o d", o=1
                                 ).broadcast_to([P, sz, D]))
            prods[c] = prod
            for g in gates_after.get(c, ()):
                nc.vector.tensor_copy(out=xs_tiles[g][0:1, 0, 0:1],
                                      in_=prod[0:1, 0, 0:1])
            if sz == 4:
                nc.vector.tensor_reduce(
                    s_o_mat[:, n0:n0 + sz], prod,
                    axis=mybir.AxisListType.X, op=mybir.AluOpType.add)
            else:
                for blk in range(sz):
                    t = n0 + blk
                    nc.scalar.activation(
                        prod[:, blk, :], prod[:, blk, :],
                        mybir.ActivationFunctionType.Copy,
                        accum_out=s_o_mat[:, t:t + 1])
        nc.vector.tensor_copy(out=xs_tiles[3][0:1, 0, 0:1],
                              in_=prods[7][0:1, 0, 0:1])
        nc.vector.tensor_copy(out=xs_tiles[4][0:1, 0, 0:1],
                              in_=prods[7][0:1, 0, 0:1])

        # simp loads: queued on Sync, released by the gate writes above
        m0 = 0
        for g, sz in enumerate(SCHUNKS):
            nc.sync.dma_start(out=xs_tiles[g],
                              in_=xs_re[:, m0:m0 + sz, :])
            m0 += sz

        # --- s_o -> fp16 row tile: PE transpose, ScalarE (idle) copies the
        # psum result to SBUF fp16 so DVE can start on the simp dots ---
        nc.tensor.transpose(tp_psum, s_o_mat, identity)
        nc.scalar.copy(t16, tp_psum)

        # --- broadcast s_o: 16 K=16 matmuls; the block-identity selector
        # stationary sel_n extracts row n of t16 onto every partition
        # (matmul operands must start at partition 0, so t16[n:n+1] cannot
        # be the moving operand directly) ---
        for n in range(NT):
            nc.tensor.matmul(sob_psum[:, n * P:(n + 1) * P],
                             sel[:, n * P:(n + 1) * P], t16,
                             start=False, stop=(n % 4 == 3))

        # --- phase 1b + 2: simp half -> s_s (chunked mul+reduce), outputs ---
        o_sb = None
        m0 = 0
        for g, ssz in enumerate(SCHUNKS):
            xs = xs_tiles[g]
            prod = spool.tile([P, ssz, D], F16, tag=f"prod{ssz}",
                              name=f"ps{g}")
            nc.vector.tensor_mul(out=prod, in0=xs,
                                 in1=w_bc[:, D:2 * D].rearrange(
                                     "p (o d) -> p o d", o=1
                                 ).broadcast_to([P, ssz, D]))
            nc.vector.tensor_reduce(
                s_sb_mat[:, m0:m0 + ssz], prod,
                axis=mybir.AxisListType.X, op=mybir.AluOpType.add)
            for blk in range(ssz):
                t = m0 + blk
                # tiles 0 and 15 ship as single 0.5 MiB stores so the output
                # stream starts one sigmoid earlier; the rest pair up
                if t in (0, NT - 1):
                    o_sb = opool.tile([P, 2, S], F16, tag="osb",
                                      name=f"osingle{t}")
                    nc.scalar.activation(
                        o_sb[:, 0, :], sob_psum,
                        mybir.ActivationFunctionType.Sigmoid,
                        bias=s_sb_mat[:, t:t + 1], scale=1.0)
                    nc.sync.dma_start(out=out[t * P:(t + 1) * P, :],
                                      in_=o_sb[:, 0, :])
                    continue
                q = (t - 1) % 2
                if q == 0:
                    o_sb = opool.tile([P, 2, S], F16, tag="osb",
                                      name=f"opair{t // 2}")
                nc.scalar.activation(
                    o_sb[:, q, :], sob_psum,
                    mybir.ActivationFunctionType.Sigmoid,
                    bias=s_sb_mat[:, t:t + 1],
                    scale=1.0,
                )
                if q == 1:
                    r0 = (t - 1) * P
                    dst = out[r0:r0 + 2 * P, :].rearrange(
                        "(q p) i -> p q i", p=P)
                    nc.sync.dma_start(out=dst, in_=o_sb)
            m0 += ssz


def build_program():
    nc = bacc.Bacc(
        "TRN2",
        debug=False,
        target_bir_lowering=False,
        num_devices=NCORES,
    )
    x = nc.dram_tensor("x", [2 * S, D], F32, kind="ExternalInput").ap()
    w = nc.dram_tensor("w", [1, 2 * D], F32, kind="ExternalInput").ap()
    bvec = nc.dram_tensor("bvec", [1, 1], F32, kind="ExternalInput").ap()
    sel_in = nc.dram_tensor("sel", [NT, S], F16, kind="ExternalInput").ap()
    out = nc.dram_tensor("out", [S, S], F16, kind="ExternalOutput").ap()
    with TileContext(nc) as tc:
        _kernel_body(tc, out, x, w, bvec, sel_in)
    nc.compile()
    return nc


_PROGRAM = None


def _get_program():
    global _PROGRAM
    if _PROGRAM is None:
        _PROGRAM = build_program()
    return _PROGRAM


def make_in_maps(prop_state, W, b):
    prop = np.ascontiguousarray(np.asarray(prop_state, dtype=np.float32))
    w = np.ascontiguousarray(np.asarray(W, dtype=np.float32).reshape(1, 2 * D))
    bv = np.ascontiguousarray(np.asarray(b, dtype=np.float32).reshape(1, 1))
    assert prop.shape == (NCORES, 2 * S, D), prop.shape
    sel = np.kron(np.eye(NT), np.ones((1, P))).astype(np.float16)
    return [{"x": prop[i], "w": w, "bvec": bv, "sel": sel}
            for i in range(NCORES)]


def kernel(A, prop_state, W, b, _trace=False):
    nc = _get_program()
    in_maps = make_in_maps(prop_state, W, b)
    res = bass_utils.run_bass_kernel_spmd(
        nc, in_maps, core_ids=list(range(NCORES)), trace=_trace)
    out = np.stack(
        [np.asarray(res.results[i]["out"], dtype=np.float32)
         for i in range(NCORES)], axis=0)
    if _trace:
        kernel.last_results = res
    return out


# revision 21
# speedup vs baseline: 1.1130x; 1.1130x over previous
"""Trainium2 Bass kernel for nn_AlignModel.

Computes out[b, j, i] = sigmoid(simp[b,j]·w_s + orig[b,i]·w_o + bias) where
orig/simp are the two halves of prop_state[b] ([B, 2S, D] -> [B,S,D] each),
w_o = W[0,:D], w_s = W[0,D:].

Sharding: data-parallel over batch B=8 across the 8 NeuronCores. Each core:
  in  x   [4096, 512] f32  (= prop_state[b])
  in  w   [1, 1024]   f32
  in  bvec[1, 1]      f32
  out out [2048, 2048] fp16 (= sigmoid(s_s[:,None] + s_o[None,:] + b)),
  upcast to f32 on the host (tolerance is 2e-2; fp16 adds ~2^-11 rel).

Structure (v5).  Measured facts driving it: HBM reads sustain ~390 GB/s
once ramped but each DMA issue costs ~0.6-1 us on its queue's engine;
concurrent DMAs in a queue drain round-robin (chunk completion order is
by SIZE, not issue order); ScalarE sigmoid is 1 elem/cycle @1.2 GHz so
the 16-tile sigmoid train is a fixed ~32 us tail; DMA completion
semaphores cost extra microseconds when the consumer is another queue's
engine.

  - fp16 output halves store traffic (8.4 MB in + 8.4 MB out).
  - Critical path: orig half loaded -> s_o -> PSUM broadcast -> sigmoid
    train.  Orig loads use INCREASING chunk sizes [1,1,2,3,4,5] so
    round-robin completion is staggered (the dot pipeline starts on
    chunk 0 early) and are split across two queues (Sync 4 issues,
    Scalar 2) so all streams are live ~3 us sooner.
  - Dots: Pool (gpsimd) does the elementwise multiply, DVE the row
    reduce -- each leg is under the ~0.8 us/tile load pace, so s_o
    completes ~1.4 us after the last orig tile lands.  (Fallback
    POOL_MUL=False: DVE muls + ScalarE activation-Copy accum for early
    tiles, DVE reduces for the last chunk.)
  - s_o_mat [128,16] -> PE transpose (via identity) -> [16,128] psum ->
    DVE copy to fp16 t16 -> PSUM broadcast by SIXTEEN 128-wide rank-1
    matmuls whose moving operand is row n of t16 directly.  No DMA on
    this chain (a DMA hop here repeatedly cost 4+ us in completion
    semaphore latency).  The bias b is PSUM-seeded early with matching
    128-wide regions (start=True; the s_o matmuls close with stop=True).
  - simp loads are WAW-gated to release right as the orig stream
    finishes; simp dots (same Pool/DVE split) stay ahead of the sigmoid
    train.
  - Each output row-tile is ONE ScalarE op
      out_t = Sigmoid(psum(b + s_o) + s_s_col_t)   (PSUM -> SBUF fp16);
    row-tiles 0 and 15 ship as single 0.5 MiB stores, the rest as 1 MiB
    pairs.
  (tensor_tensor_reduce would fuse mul+reduce in one DVE pass but
  crashes the exec unit on real TRN2 despite passing CoreSim.)
"""

import numpy as np

import concourse.mybir as mybir
from concourse import bacc, bass_utils, masks
from concourse.tile import TileContext

P = 128          # partitions
D = 512          # feature dim
S = 2048         # sents
NT = S // P      # 16 tiles per half
OCHUNKS = [1, 1, 2, 4, 4, 4]   # orig tiles per chunk, staggered completion
SCHUNKS = [1, 3, 4, 4, 4]      # simp tiles per chunk
NCORES = 8
F32 = mybir.dt.float32
F16 = mybir.dt.float16
# NOTE: Pool (gpsimd) tensor_mul and DVE tensor_tensor_reduce both crash
# the exec unit on real TRN2 inside this kernel (they pass CoreSim and
# work standalone) -- only memset/affine_select/DMA run on Pool here.


def _kernel_body(tc, out, x, w, bvec, sel_in):
    nc = tc.nc
    # both halves partition-inner: row index = n*P + p
    xo_re = x[0:S, :].rearrange("(n p) d -> p n d", p=P)
    xs_re = x[S:2 * S, :].rearrange("(n p) d -> p n d", p=P)
    with (
        tc.tile_pool(name="consts", bufs=1) as cpool,
        tc.tile_pool(name="xin", bufs=1) as xpool,
        tc.tile_pool(name="scratch", bufs=8) as spool,
        tc.tile_pool(name="outbuf", bufs=4) as opool,
        tc.tile_pool(name="psum", bufs=1, space="PSUM") as ppool,
    ):
        # b first: 4 bytes, completes immediately, unblocks the PSUM b-seed
        b_sb = cpool.tile([1, 1], F32, tag="bsb")
        nc.sync.dma_start(out=b_sb, in_=bvec)

        # --- orig input stream: geometric chunks, all on the Sync queue
        # (the DMA fabric is one ~390 GB/s pool; multiple queues just
        # split it).  Increasing sizes stagger the round-robin completion
        # order so the dot pipeline starts on chunk 0 early. ---
        xo_tiles = []
        n0 = 0
        for c, sz in enumerate(OCHUNKS):
            xo = xpool.tile([P, sz, D], F32, tag=f"xo{c}", name=f"xo{c}")
            nc.sync.dma_start(out=xo, in_=xo_re[:, n0:n0 + sz, :])
            xo_tiles.append(xo)
            n0 += sz

        # simp tiles; loads issued later on the Sync queue, WAW-gated
        xs_tiles = [
            xpool.tile([P, sz, D], F32, tag=f"xs{g}", name=f"xs{g}")
            for g, sz in enumerate(SCHUNKS)
        ]

        # w / sel replicated via the Pool queue (idle); w_o first since it
        # gates the first mul.
        w_bc = cpool.tile([P, 2 * D], F32, tag="wbc")
        nc.gpsimd.dma_start(out=w_bc[:, 0:D],
                            in_=w[:, 0:D].broadcast_to([P, D]))
        nc.gpsimd.dma_start(out=w_bc[:, D:2 * D],
                            in_=w[:, D:2 * D].broadcast_to([P, D]))
        sel = cpool.tile([NT, S], F16, tag="sel")
        nc.gpsimd.dma_start(out=sel, in_=sel_in)
        ones_row = cpool.tile([1, P], F16, tag="ones")
        nc.gpsimd.memset(ones_row, 1.0)
        identity = cpool.tile([P, P], F32, tag="ident")
        masks.make_identity(nc, identity)

        s_o_mat = cpool.tile([P, NT], F32, tag="somat")   # s_o[n*128+p] @ [p,n]
        s_sb_mat = cpool.tile([P, NT], F32, tag="ssmat")  # s_s col t
        t16 = cpool.tile([NT, P], F16, tag="t16")         # s_o transposed
        b_row = cpool.tile([1, 512], F16, tag="brow")
        nc.vector.memset(b_row, 0.0)
        nc.vector.tensor_scalar_add(b_row, b_row, b_sb)
        sob_psum = ppool.tile([P, S], F32, tag="sob")     # b + s_o, all rows
        tp_psum = ppool.tile([NT, P], F32, tag="tp")      # transpose scratch

        # dummy sigmoid: pulls the ACT table load off the critical path
        dummy = cpool.tile([1, 2], F32, tag="dummy")
        nc.vector.memset(dummy, 0.0)
        nc.scalar.activation(dummy[:, 1:2], dummy[:, 0:1],
                             mybir.ActivationFunctionType.Sigmoid)

        # --- psum b seed: one start=True matmul per PSUM bank (512 cols;
        # accumulation groups are bank-granular) ---
        for j in range(S // 512):
            nc.tensor.matmul(sob_psum[:, j * 512:(j + 1) * 512], ones_row,
                             b_row, start=True, stop=False)

        # --- phase 1a: orig half -> s_o.  DVE muls (fp16 product) pace
        # with the load stream; ScalarE -- idle until the sigmoid train --
        # accumulates tiles 0-11 via activation-Copy accum (~1.0us/tile),
        # DVE sweeps the last chunk itself right after its muls.  8 prod
        # buffers so mul t+8 never WAR-stalls on ScalarE's accum of t. ---
        gates = {12: (0,), 13: (1,), 14: (2,), 15: (3, 4)}
        late_prods = {}
        n0 = 0
        for c, sz in enumerate(OCHUNKS):
            xo = xo_tiles[c]
            for blk in range(sz):
                t = n0 + blk
                prod = spool.tile([P, D], F16, tag="prod", name=f"po{t}")
                nc.vector.tensor_mul(out=prod, in0=xo[:, blk, :],
                                     in1=w_bc[:, 0:D])
                for g in gates.get(t, ()):
                    nc.vector.tensor_copy(out=xs_tiles[g][0:1, 0, 0:1],
                                          in_=prod[0:1, 0:1])
                if t < 12:
                    nc.scalar.activation(
                        prod, prod, mybir.ActivationFunctionType.Copy,
                        accum_out=s_o_mat[:, t:t + 1])
                else:
                    late_prods[t] = prod
            n0 += sz
        for t in range(12, NT):
            nc.vector.tensor_reduce(
                s_o_mat[:, t:t + 1], late_prods[t],
                axis=mybir.AxisListType.X, op=mybir.AluOpType.add)

        # simp loads: queued on Sync, released by the gate writes above
        m0 = 0
        for g, sz in enumerate(SCHUNKS):
            nc.sync.dma_start(out=xs_tiles[g],
                              in_=xs_re[:, m0:m0 + sz, :])
            m0 += sz

        # --- s_o -> fp16 row tile: PE transpose; ScalarE (idle between
        # its accums and the train) copies the psum result to SBUF fp16
        # so DVE can go straight to the simp dots ---
        nc.tensor.transpose(tp_psum, s_o_mat, identity)
        nc.scalar.copy(t16, tp_psum)

        # --- broadcast s_o: 16 K=16 matmuls; the block-identity selector
        # stationary sel_n extracts row n of t16 onto every partition
        # (matmul operands must start at partition 0, so t16[n:n+1] cannot
        # be the moving operand directly) ---
        for n in range(NT):
            nc.tensor.matmul(sob_psum[:, n * P:(n + 1) * P],
                             sel[:, n * P:(n + 1) * P], t16,
                             start=False, stop=(n % 4 == 3))

        # --- phase 1b + 2: simp half -> s_s, then outputs ---
        o_sb = None
        m0 = 0
        for g, ssz in enumerate(SCHUNKS):
            xs = xs_tiles[g]
            for blk in range(ssz):
                t = m0 + blk
                prod = spool.tile([P, D], F16, tag="prod", name=f"ps{t}")
                nc.vector.tensor_mul(out=prod, in0=xs[:, blk, :],
                                     in1=w_bc[:, D:2 * D])
                nc.vector.tensor_reduce(
                    s_sb_mat[:, t:t + 1], prod,
                    axis=mybir.AxisListType.X, op=mybir.AluOpType.add)
            for blk in range(ssz):
                t = m0 + blk
                # tiles 0 and 15 ship as single 0.5 MiB stores so the output
                # stream starts one sigmoid earlier; the rest pair up
                if t in (0, NT - 1):
                    o_sb = opool.tile([P, 2, S], F16, tag="osb",
                                      name=f"osingle{t}")
                    nc.scalar.activation(
                        o_sb[:, 0, :], sob_psum,
                        mybir.ActivationFunctionType.Sigmoid,
                        bias=s_sb_mat[:, t:t + 1], scale=1.0)
                    nc.sync.dma_start(out=out[t * P:(t + 1) * P, :],
                                      in_=o_sb[:, 0, :])
                    continue
                q = (t - 1) % 2
                if q == 0:
                    o_sb = opool.tile([P, 2, S], F16, tag="osb",
                                      name=f"opair{t // 2}")
                nc.scalar.activation(
                    o_sb[:, q, :], sob_psum,
                    mybir.ActivationFunctionType.Sigmoid,
                    bias=s_sb_mat[:, t:t + 1],
                    scale=1.0,
                )
                if q == 1:
                    r0 = (t - 1) * P
                    dst = out[r0:r0 + 2 * P, :].rearrange(
                        "(q p) i -> p q i", p=P)
                    nc.sync.dma_start(out=dst, in_=o_sb)
            m0 += ssz


def build_program():
    nc = bacc.Bacc(
        "TRN2",
        debug=False,
        target_bir_lowering=False,
        num_devices=NCORES,
    )
    x = nc.dram_tensor("x", [2 * S, D], F32, kind="ExternalInput").ap()
    w = nc.dram_tensor("w", [1, 2 * D], F32, kind="ExternalInput").ap()
    bvec = nc.dram_tensor("bvec", [1, 1], F32, kind="ExternalInput").ap()
    sel_in = nc.dram_tensor("sel", [NT, S], F16, kind="ExternalInput").ap()
    out = nc.dram_tensor("out", [S, S], F16, kind="ExternalOutput").ap()
    with TileContext(nc) as tc:
        _kernel_body(tc, out, x, w, bvec, sel_in)
    nc.compile()
    return nc


_PROGRAM = None


def _get_program():
    global _PROGRAM
    if _PROGRAM is None:
        _PROGRAM = build_program()
    return _PROGRAM


def make_in_maps(prop_state, W, b):
    prop = np.ascontiguousarray(np.asarray(prop_state, dtype=np.float32))
    w = np.ascontiguousarray(np.asarray(W, dtype=np.float32).reshape(1, 2 * D))
    bv = np.ascontiguousarray(np.asarray(b, dtype=np.float32).reshape(1, 1))
    assert prop.shape == (NCORES, 2 * S, D), prop.shape
    sel = np.kron(np.eye(NT), np.ones((1, P))).astype(np.float16)
    return [{"x": prop[i], "w": w, "bvec": bv, "sel": sel}
            for i in range(NCORES)]


def kernel(A, prop_state, W, b, _trace=False):
    nc = _get_program()
    in_maps = make_in_maps(prop_state, W, b)
    res = bass_utils.run_bass_kernel_spmd(
        nc, in_maps, core_ids=list(range(NCORES)), trace=_trace)
    out = np.stack(
        [np.asarray(res.results[i]["out"], dtype=np.float32)
         for i in range(NCORES)], axis=0)
    if _trace:
        kernel.last_results = res
    return out
